# revision 10
# baseline (speedup 1.0000x reference)
"""Trainium2 Bass kernel for the 3-expert MoE routing MLP.

Reference computation (B=1M rows):
    y1  = tanh(x @ w1 - b1)                     # [B, 8]
    h_k = sigmoid(y1 @ wa_k - ba_k)             # [B, 16] for experts k=0,1,2
    e_k = h_k @ wb_k - bb_k                     # [B, 32]
    y   = e_{u[b]}  per row b

Device mapping (pure data parallel over 8 cores, B_C = 125000 rows/core):
  * Host packs each core's x shard transposed two-row-blocks-deep:
    x2 [128, B_H] with partitions 0-63 = x[:B_H].T and 64-127 = x[B_H:].T,
    so every PE matmul streams two batch-halves at once (B_H = B_C/2).
  * All weights are packed host-side into three block-diagonal lhsT
    operands so the whole per-row pipeline is 3 matmuls + 2 activations
    + 1 compare + 1 PSUM->SBUF copy:
      trunk:  P_t[16,n]  = W_tr.T @ x2          -> y1t = tanh(P_t - b1)
      H:      P_h[96,n]  = W_h.T @ [y1t; oh]    (adds +BIG on selected
              expert rows via onehot), G = sigmoid(P_h - ba - BIG)
              => G rows of non-selected experts ~ 0 (mask via saturation)
      final:  yT[64,n]   = W_f.T @ [oh; G]      (wb blocks + (-bb) rows)
  * onehot oh[6,n] = (u6 == [0,1,2,0,1,2]) computed on DVE from a
    host-replicated int32 u6 [6, B_H].
  * Output is written transposed ([64, B_H] per core) for contiguous DMA;
    the host unpacks back to [B, 32].
"""

import math

import numpy as np

import concourse.bass as bass
import concourse.tile as tile
from concourse import mybir
from concourse.bass_utils import run_bass_kernel_spmd

F32 = mybir.dt.float32
F32R = mybir.dt.float32r
I32 = mybir.dt.int32

N_CORES = 8
B = 1_000_000
IN = 64
OUT = 32
B_C = B // N_CORES          # rows per core
B_H = B_C // 2              # packed free length per core
BIG = 40.0

F_TILE = 2000               # SBUF tile free size
CHUNK = 500                 # PSUM matmul free size (<=512 fp32)

MM_DT = F32R                # matmul compute dtype (fp32 data, fast mode)


def _pack_weights(w1, b1, w2, b2, w3, b3, w4, b4, w5, b5, w6, b6, w7, b7):
    f32 = np.float32
    W_tr = np.zeros((128, 16), f32)
    W_tr[0:64, 0:8] = w1
    W_tr[64:128, 8:16] = w1

    wa_all = np.concatenate([w2, w4, w6], axis=1)        # [8, 48]
    # H lhsT is split: W_ha streams y1t [16 rows], W_hb streams onehot [6
    # rows]; both accumulate into the same [102, n] PSUM tile.  G rows
    # 0-95 are the expert hiddens (two batch halves), rows 96-101 are
    # pseudo-mask units: sigmoid(BIG*oh - BIG/2) ~= oh, used to apply the
    # per-row -bb_sel in the final matmul.
    W_ha = np.zeros((16, 102), f32)
    W_ha[0:8, 0:48] = wa_all
    W_ha[8:16, 48:96] = wa_all
    W_hb = np.zeros((6, 102), f32)
    for k in range(3):
        W_hb[k, 16 * k:16 * (k + 1)] = BIG
        W_hb[3 + k, 48 + 16 * k:48 + 16 * (k + 1)] = BIG
        W_hb[k, 96 + k] = BIG
        W_hb[3 + k, 99 + k] = BIG

    wb_all = np.concatenate([w3, w5, w7], axis=0)        # [48, 32]
    bb = [b3, b5, b7]
    W_f = np.zeros((102, 64), f32)
    W_f[0:48, 0:32] = wb_all
    W_f[48:96, 32:64] = wb_all
    for k in range(3):
        W_f[96 + k, 0:32] = -bb[k]
        W_f[99 + k, 32:64] = -bb[k]

    ba_all = np.concatenate([b2, b4, b6])                # [48]
    bias_t = np.concatenate([-b1, -b1]).astype(f32)
    bias_h = np.concatenate(
        [-ba_all - BIG, -ba_all - BIG, np.full(6, -BIG / 2, f32)]).astype(f32)
    kvec = np.array([0, 1, 2, 0, 1, 2], np.float32)

    # Single packed weight tensor (one DMA -> one semaphore lane):
    # cols 0:16 W_tr [128r], 16:118 W_ha [16r], 118:220 W_hb [6r],
    # 220:284 W_f [102r].
    wpack = np.zeros((128, 284), f32)
    wpack[:, 0:16] = W_tr
    wpack[0:16, 16:118] = W_ha
    wpack[0:6, 118:220] = W_hb
    wpack[0:102, 220:284] = W_f
    # bpack [102, 3]: col 0 bias_t (rows 0:16), col 1 bias_h, col 2 kvec.
    bpack = np.zeros((102, 3), f32)
    bpack[0:16, 0] = bias_t
    bpack[0:102, 1] = bias_h
    bpack[0:6, 2] = kvec
    return dict(wpack=wpack, bpack=bpack)


def _split_multi_waits(nc):
    """Walrus codegen allows one sync-wait per instruction; hoist extra
    waits onto same-engine NoOps inserted just before the instruction."""
    n = 0
    for fn in nc.m.functions:
        for blk in fn.blocks:
            out = []
            for ins in blk.instructions:
                si = ins.sync_info
                if si is not None and len(si.on_wait) > 1:
                    waits = list(si.on_wait)
                    for j, w in enumerate(waits[:-1]):
                        nop = mybir.InstNoOp(name=f"{ins.name}-wsplit{j}")
                        nop.engine = ins.engine
                        nop.sync_info = mybir.SyncInfo(on_wait=[w],
                                                       on_update=[])
                        nc.register_instruction(nop)
                        out.append(nop)
                        n += 1
                    si.on_wait = [waits[-1]]
                out.append(ins)
            blk.instructions[:] = out
    return n


def build_nc(b_h=B_H, f_tile=F_TILE, chunk=CHUNK):
    nc = bass.Bass("TRN2", target_bir_lowering=False, debug=False)

    x2_d = nc.dram_tensor("x2", [128, b_h], F32, kind="ExternalInput").ap()
    u6_d = nc.dram_tensor("u6", [6, b_h], I32, kind="ExternalInput").ap()
    wp_d = nc.dram_tensor("wpack", [128, 284], F32, kind="ExternalInput").ap()
    bp_d = nc.dram_tensor("bpack", [102, 3], F32, kind="ExternalInput").ap()
    yt_d = nc.dram_tensor("yT", [64, b_h], F32, kind="ExternalOutput").ap()

    n_tiles = math.ceil(b_h / f_tile)

    with tile.TileContext(nc) as tc:
        with (
            tc.tile_pool(name="const", bufs=1) as cpool,
            tc.tile_pool(name="xin", bufs=3) as xpool,
            tc.tile_pool(name="uin", bufs=3) as upool,
            tc.tile_pool(name="ty", bufs=2) as ypool,
            tc.tile_pool(name="toh", bufs=2) as ohpool,
            tc.tile_pool(name="tg", bufs=2) as gpool,
            tc.tile_pool(name="outp", bufs=3) as opool,
            tc.tile_pool(name="pt", bufs=2, space="PSUM") as ptp,
            tc.tile_pool(name="ph", bufs=2, space="PSUM") as php,
            tc.tile_pool(name="pf", bufs=4, space="PSUM") as pfp,
        ):
            wp = cpool.tile([128, 284], MM_DT)
            nc.sync.dma_start(wp[:], wp_d.bitcast(MM_DT))
            W_tr = wp[:, 0:16]
            W_ha = wp[0:16, 16:118]
            W_hb = wp[0:6, 118:220]
            W_f = wp[0:102, 220:284]
            bp = cpool.tile([102, 3], F32)
            nc.sync.dma_start(bp[:], bp_d)
            b_t = bp[0:16, 0:1]
            b_h_t = bp[0:102, 1:2]
            kv = bp[0:6, 2:3]

            for it in range(n_tiles):
                f0 = it * f_tile
                fs = min(f_tile, b_h - f0)

                xt = xpool.tile([128, f_tile], MM_DT)
                nc.sync.dma_start(xt[:, :fs], x2_d[:, f0:f0 + fs].bitcast(MM_DT))
                ut = upool.tile([6, f_tile], I32)
                nc.sync.dma_start(ut[:, :fs], u6_d[:, f0:f0 + fs])

                T_y = ypool.tile([16, f_tile], MM_DT)
                T_oh = ohpool.tile([6, f_tile], MM_DT)
                T_g = gpool.tile([102, f_tile], MM_DT)
                nc.vector.tensor_scalar(
                    T_oh[:, :fs], ut[:, :fs], kv[:], None,
                    mybir.AluOpType.is_equal,
                )

                ot = opool.tile([64, f_tile], F32)

                n_chunks = math.ceil(fs / chunk)
                for c in range(n_chunks):
                    c0 = c * chunk
                    cs = min(chunk, fs - c0)
                    cc = slice(c0, c0 + cs)

                    p_t = ptp.tile([16, chunk], F32, tag="p_t")
                    nc.tensor.matmul(p_t[:, :cs], W_tr[:], xt[:, cc],
                                     start=True, stop=True)
                    nc.scalar.activation(
                        T_y[:, cc], p_t[:, :cs],
                        mybir.ActivationFunctionType.Tanh,
                        bias=b_t[:], scale=1.0)

                    p_h = php.tile([102, chunk], F32)
                    nc.tensor.matmul(p_h[:, :cs], W_ha[:],
                                     T_y[:, cc],
                                     start=True, stop=False)
                    nc.tensor.matmul(p_h[:, :cs], W_hb[:],
                                     T_oh[:, cc],
                                     start=False, stop=True)
                    nc.scalar.activation(
                        T_g[:, cc], p_h[:, :cs],
                        mybir.ActivationFunctionType.Sigmoid,
                        bias=b_h_t[:], scale=1.0)

                    p_f = pfp.tile([64, chunk], F32)
                    nc.tensor.matmul(p_f[:, :cs], W_f[:],
                                     T_g[:, cc],
                                     start=True, stop=True)
                    nc.vector.tensor_copy(ot[:, cc], p_f[:, :cs])

                nc.sync.dma_start(yt_d[:, f0:f0 + fs], ot[:, :fs])

    _split_multi_waits(nc)
    return nc


_NC_CACHE = {}


def _get_nc(b_h=B_H, f_tile=F_TILE, chunk=CHUNK):
    key = (b_h, f_tile, chunk)
    if key not in _NC_CACHE:
        _NC_CACHE[key] = build_nc(*key)
    return _NC_CACHE[key]


def make_in_maps(x, u, weights, n_cores=N_CORES):
    """Shard + pack full inputs into per-core in_maps."""
    packed = _pack_weights(*weights)
    b = x.shape[0]
    b_c = b // n_cores
    b_h = b_c // 2
    in_maps = []
    for c in range(n_cores):
        xc = x[c * b_c:(c + 1) * b_c]
        uc = u[c * b_c:(c + 1) * b_c]
        x2 = np.empty((128, b_h), np.float32)
        x2[0:64] = xc[:b_h].T
        x2[64:128] = xc[b_h:].T
        u6 = np.empty((6, b_h), np.int32)
        u6[0:3] = uc[:b_h]
        u6[3:6] = uc[b_h:]
        in_maps.append({"x2": x2, "u6": u6, **packed})
    return in_maps


def unpack_outputs(results, n_cores=N_CORES):
    b_h = results[0]["yT"].shape[1]
    b_c = 2 * b_h
    y = np.empty((n_cores * b_c, OUT), np.float32)
    for c in range(n_cores):
        yt = results[c]["yT"]
        y[c * b_c:c * b_c + b_h] = yt[0:32].T
        y[c * b_c + b_h:(c + 1) * b_c] = yt[32:64].T
    return y


def kernel(x, u, w1, b1, w2, b2, w3, b3, w4, b4, w5, b5, w6, b6, w7, b7):
    x = np.ascontiguousarray(np.asarray(x, np.float32))
    u = np.ascontiguousarray(np.asarray(u, np.int32))
    weights = [np.asarray(t, np.float32) for t in
               (w1, b1, w2, b2, w3, b3, w4, b4, w5, b5, w6, b6, w7, b7)]

    nc = _get_nc()
    in_maps = make_in_maps(x, u, weights)
    res = run_bass_kernel_spmd(nc, in_maps, core_ids=list(range(N_CORES)))
    return unpack_outputs(res.results)
